# revision 1
# baseline (speedup 1.0000x reference)
"""Trainium2 Bass kernel for Falcon-7B MQA flash-decode attention block.

Geometry (hardcoded from the problem spec):
  hidden [1, 32, 4544], w_qkv [4672, 4544] (71 q heads + 1 k + 1 v, hd=64),
  kv cache [4, 1, 32, 2048, 64], masks [4, 1, 32, 2048], w_dense [4544, 4544].

Sharding across 8 NeuronCores:
  - users (32) are data-parallel, 4 per core: each core holds its users' KV.
  - w_qkv / w_dense are tensor-parallel column-split 8 ways; an AllToAll
    redistributes the fused QKV activations from column-shards to user-shards,
    and wave-split AllGathers collect attention outputs for the dense matmul
    while later users are still computing.
  - softmax uses the shift-invariant (max-free) formulation, which is exact
    for these magnitudes in fp32; masks enter through the ACT exp bias.

Host-side prep is layout-only (transposes / slicing / padding of inputs).
"""

import os
import sys

if "/opt/trn_rl_repo" not in sys.path:
    sys.path.insert(0, "/opt/trn_rl_repo")

import numpy as np

import concourse.bacc as bacc
import concourse.bass as bass
import concourse.mybir as mybir
import concourse.tile as tile
from concourse.bass_utils import run_bass_kernel_spmd
from concourse.masks import make_identity

F32 = mybir.dt.float32
# float32r: same fp32 bits, ~4x faster moving operand at free-dim >= 256, but
# hardware-measured relative error grows to ~3e-4 (vs 1.5e-5 pure fp32).
# Both weight-matmul phases are DMA-bound here, so fp32 is the default.
F32R = mybir.dt.float32r
WDT = F32R if os.environ.get("F32R", "0") == "1" else F32

NCORES = 8
U = 32          # users total
UPC = 4         # users per core
HID = 4544
NH = 71         # query heads
HD = 64
HPC = 10        # heads per core in the padded qkv column split (8*10*64 = 5120)
NCOL = HPC * HD         # 640 fused columns per core
DN = HID // NCORES      # 568 dense output columns per core
S = 8192                # total cached tokens per user (4 chunks x 2048)
NT = S // 128           # 64 s-tiles of 128
NTH = NT // 2           # 32 tiles per kT partition-half
KT = 36                 # k-tiles over HID: 35 x 128 + 1 x 64
KTG = 6                 # k-tiles per attnT group, slab-aligned (6 groups)
ROWS_FULL = 35 * 128    # 4480
WAVE_USERS = (3, 1)     # attn AllGather wave sizes (users 0-2, then user 3)

LAST_RESULT = None
_prog = None


def _build():
    nc = bacc.Bacc("TRN2", target_bir_lowering=False, debug=False,
                   num_devices=NCORES)

    hT = nc.dram_tensor("hT", [HID, U], WDT, kind="ExternalInput")
    wq = nc.dram_tensor("wq", [HID, NCOL], WDT, kind="ExternalInput")
    wd = nc.dram_tensor("wd", [HID, DN], WDT, kind="ExternalInput")
    kTc = nc.dram_tensor("kTc", [UPC, 128, S // 2], F32, kind="ExternalInput")
    vc = nc.dram_tensor("vc", [UPC, S, HD], F32, kind="ExternalInput")
    mc = nc.dram_tensor("mc", [UPC, NT, 128], F32, kind="ExternalInput")
    # MuT[i] = (diag(cos_u) + diag(sin_u) @ R)^T per local user, R = rotate_half
    muT = nc.dram_tensor("muT", [HD, UPC, HD], F32, kind="ExternalInput")
    outc = nc.dram_tensor("outc", [U, DN], F32, kind="ExternalOutput")

    with tile.TileContext(nc) as tc:
        with (
            tc.tile_pool(name="const", bufs=1) as const,
            tc.tile_pool(name="wpool", bufs=2) as wpool,
            tc.tile_pool(name="wdpool", bufs=6) as wdpool,
            tc.tile_pool(name="kvpool", bufs=2) as kvpool,
            tc.tile_pool(name="upool", bufs=2) as upool,
            tc.tile_pool(name="ppool", bufs=2) as ppool,
            tc.tile_pool(name="pspool", bufs=1, space="PSUM") as pspool,
            tc.tile_pool(name="ps4pool", bufs=2, space="PSUM") as ps4pool,
            tc.tile_pool(name="pvpool", bufs=1, space="PSUM") as pvpool,
            tc.tile_pool(name="pstpool", bufs=2, space="PSUM") as pstpool,
            tc.tile_pool(name="dram", bufs=1, space="DRAM") as dram,
        ):
            identity = const.tile([128, 128], F32)
            make_identity(nc, identity)

            # ---------------- phase A: fused QKV projection ----------------
            hT_all = const.tile([128, KT, U], WDT)
            nc.sync.dma_start(
                out=hT_all[:, 0:35, :],
                in_=hT[0:ROWS_FULL, :].rearrange("(t p) u -> p t u", p=128))
            nc.sync.dma_start(out=hT_all[0:64, 35, :], in_=hT[ROWS_FULL:HID, :])

            muT_sb = const.tile([HD, UPC, HD], F32)
            nc.sync.dma_start(out=muT_sb, in_=muT[:, :, :])

            # 4 concurrent col-group matmuls: col-group j computes fused
            # columns 160j..160j+159 for all 32 users on psum partitions 32j+
            QC = NCOL // 4  # 160
            psQ = pspool.tile([128, QC], F32, tag="bank", name="psQ")
            for g in range(7):
                wslab = wpool.tile([128, 5, NCOL], WDT, tag="w", name="wslab")
                if g == 0:
                    # split the first slab so the projection can start after
                    # one k-tile (128 rows) instead of the full 1.6 MB slab
                    nc.sync.dma_start(
                        out=wslab[:, 0:1, :],
                        in_=wq[0:128, :].rearrange("(t p) n -> p t n", p=128))
                    nc.sync.dma_start(
                        out=wslab[:, 1:5, :],
                        in_=wq[128:640, :].rearrange("(t p) n -> p t n",
                                                     p=128))
                else:
                    nc.sync.dma_start(
                        out=wslab,
                        in_=wq[g * 640:(g + 1) * 640, :].rearrange(
                            "(t p) n -> p t n", p=128))
                for t5 in range(5):
                    t = 5 * g + t5
                    lhs = hT_all[:, t, :]
                    for j in range(4):
                        nc.tensor.matmul(
                            psQ[32 * j:32 * j + 32, :], lhs,
                            wslab[:, t5, QC * j:QC * (j + 1)],
                            start=(t == 0), stop=False,
                            tile_position=(0, 32 * j))
            wlast = wpool.tile([64, NCOL], WDT, tag="wl", name="wlast")
            nc.sync.dma_start(out=wlast, in_=wq[ROWS_FULL:HID, :])
            for j in range(4):
                nc.tensor.matmul(psQ[32 * j:32 * j + 32, :],
                                 hT_all[0:64, 35, :],
                                 wlast[:, QC * j:QC * (j + 1)],
                                 start=False, stop=True,
                                 tile_position=(0, 32 * j))

            # ACT does this copy: the DVE queue must stay free for the
            # first user's small copies (head-of-line blocking otherwise)
            fq_sb = const.tile([128, QC], F32)
            nc.scalar.copy(out=fq_sb, in_=psQ[:, :])

            fused_x = dram.tile([U, NCOL], F32)
            fused_x_ji = bass.AP(
                tensor=fused_x.tensor, offset=fused_x.offset,
                ap=[[QC, 4], [NCOL, U], [1, QC]])
            nc.sync.dma_start(out=fused_x_ji, in_=fq_sb)
            # block d of the flat input (users 4d..4d+3) goes to core d
            fused_loc = dram.tile([NCORES, UPC, NCOL], F32)
            nc.gpsimd.collective_compute(
                "AllToAll", mybir.AluOpType.bypass,
                replica_groups=[list(range(NCORES))],
                ins=[fused_x.opt()], outs=[fused_loc.opt()])

            # batched gathers for all 4 local users (few large-ish DMAs
            # instead of many tiny serialized ones)
            q_all = const.tile([80, UPC, HD], F32)      # (head, user, d)
            for c in range(NCORES):
                nc.sync.dma_start(
                    out=q_all[c * HPC:(c + 1) * HPC, :, :],
                    in_=fused_loc[c, :, :].rearrange("i (h d) -> h i d", d=HD))
            vcur_all = const.tile([1, UPC, HD + 1], F32)  # [v_cur | 1]
            nc.sync.dma_start(
                out=vcur_all[:, :, 0:HD],
                in_=fused_loc[7, :, 2 * HD:3 * HD][None, :, :])
            nc.vector.memset(vcur_all[:, :, HD:HD + 1], 1.0)
            mask_all = const.tile([NT, UPC, 128], F32)
            nc.sync.dma_start(
                out=mask_all,
                in_=mc.rearrange("i t p -> t i p"))

            # ---------------- phase C: per-user flash-decode attention ------
            attn_cw = [dram.tile([WAVE_USERS[w], HID], F32,
                                 name=f"attn_c{w}", uniquify=True)
                       for w in range(2)]
            attn_agw = [dram.tile([NCORES, WAVE_USERS[w], HID], F32,
                                  addr_space="Shared", name=f"attn_ag{w}",
                                  uniquify=True) for w in range(2)]

            wd_slabs = []

            def _emit_wd_slab(g):
                # 2 k-tiles per slab, 17 slabs cover tiles 0..33
                wdslab = wdpool.tile([128, 2, DN], WDT, tag="w",
                                     name="wdslab", uniquify=True)
                nc.sync.dma_start(
                    out=wdslab,
                    in_=wd[g * 256:(g + 1) * 256, :].rearrange(
                        "(t p) n -> p t n", p=128))
                wd_slabs.append(wdslab)

            for i in range(UPC):
                kT_sb = kvpool.tile([128, S // 2], F32, tag="kT", name="kT_sb")
                nc.sync.dma_start(out=kT_sb, in_=kTc[i])
                vones = kvpool.tile([128, NT, HD + 1], F32, tag="v",
                                    name="vones")
                nc.sync.dma_start(
                    out=vones[:, :, 0:HD],
                    in_=vc[i].rearrange("(t p) d -> p t d", p=128))
                nc.vector.memset(vones[:, :, HD:HD + 1], 1.0)

                ps_m = pstpool.tile([128, NT], F32, tag="pst", name="ps_m")
                nc.tensor.transpose(ps_m, mask_all[:, i, :],
                                    identity[0:NT, 0:NT])
                # expm[:, j] = exp(mask of s-tile j); p = exp(s/8) * expm
                # (exact for zero masks, ~1 ulp otherwise)
                expm = upool.tile([128, NT], F32, tag="msb", name="expm")
                nc.scalar.activation(out=expm, in_=ps_m,
                                     func=mybir.ActivationFunctionType.Exp)

                # q heads 0..70 plus the shared k head at row 71, transposed
                ps_qT = pstpool.tile([HD, NH + 1], F32, tag="pst",
                                     name="ps_qT")
                nc.tensor.transpose(ps_qT, q_all[0:NH + 1, i, :],
                                    identity[0:NH + 1, 0:NH + 1])
                qkT = upool.tile([HD, NH + 1], F32, tag="qkT", name="qkT")
                nc.vector.tensor_copy(out=qkT, in_=ps_qT)

                # rotary as a matmul; duplicated to partitions 64..127 so the
                # second kT half can use it as a same-base moving operand
                ps_rot = pstpool.tile([128, NH + 1], F32, tag="pst",
                                      name="ps_rot")
                nc.tensor.matmul(ps_rot[0:64, :], muT_sb[:, i, :], qkT,
                                 start=True, stop=True)
                nc.tensor.matmul(ps_rot[64:128, :], muT_sb[:, i, :], qkT,
                                 start=True, stop=True)
                qTr = upool.tile([128, NH + 1], F32, tag="qTr", name="qTr")
                nc.vector.tensor_copy(out=qTr, in_=ps_rot)

                # scores^T + exp for all 64 s-tiles. Tiles are emitted in
                # half-interleaved order (seq) so the two PE row-groups run
                # concurrently; pT slot s holds tile seq[s]. Exps are batched
                # 4 tiles per ACT op; the mask enters as an exp(mask)
                # multiply on the otherwise-idle DVE.
                pT_all = ppool.tile([128, NT, NH], F32, tag="pT",
                                    name="pT_all")
                seq = []
                for jp in range(NTH):
                    seq += [jp, jp + NTH]
                for b in range(NT // 2):
                    js = seq[2 * b:2 * b + 2]
                    # one matmul per PSUM bank (free-dim stride 512)
                    ps4 = ps4pool.tile([128, 2, 512], F32, tag="s4",
                                       name="ps4")
                    for idx, j in enumerate(js):
                        if j < NTH:
                            lhsT = kT_sb[0:64, j * 128:(j + 1) * 128]
                            rhs = qTr[0:64, 0:NH]
                        else:
                            lhsT = kT_sb[64:128,
                                         (j - NTH) * 128:(j - NTH + 1) * 128]
                            rhs = qTr[64:128, 0:NH]
                        nc.tensor.matmul(ps4[:, idx, 0:NH], lhsT, rhs,
                                         start=True, stop=True)
                    tmp4 = upool.tile([128, 2, NH], F32, tag="tmp4",
                                      name="tmp4")
                    nc.scalar.activation(
                        out=tmp4, in_=ps4[:, :, 0:NH],
                        func=mybir.ActivationFunctionType.Exp, scale=0.125)
                    for idx, j in enumerate(js):
                        nc.vector.tensor_scalar_mul(
                            pT_all[:, 2 * b + idx, :], tmp4[:, idx, :],
                            expm[:, j:j + 1])

                # current-token score for all heads: [1, 71]
                ps_sc = pstpool.tile([1, NH], F32, tag="pst", name="ps_sc")
                nc.tensor.matmul(ps_sc, qTr[0:64, NH:NH + 1], qTr[0:64, 0:NH],
                                 start=True, stop=True)
                curw = upool.tile([1, NH], F32, tag="curw", name="curw")
                nc.scalar.activation(out=curw, in_=ps_sc,
                                     func=mybir.ActivationFunctionType.Exp,
                                     scale=0.125)

                # PV with fused row-sum via the ones column
                pv = pvpool.tile([NH, HD + 1], F32, tag="pv", name="pv")
                for s in range(NT):
                    nc.tensor.matmul(pv, pT_all[:, s, :],
                                     vones[:, seq[s], :],
                                     start=(s == 0), stop=False)
                nc.tensor.matmul(pv, curw, vcur_all[:, i, :], start=False,
                                 stop=True)

                linv = upool.tile([NH, 1], F32, tag="linv", name="linv")
                nc.vector.reciprocal(out=linv, in_=pv[:, HD:HD + 1])
                attn_sb = upool.tile([NH, HD], F32, tag="attn", name="attn_sb")
                nc.vector.tensor_scalar_mul(attn_sb, pv[:, 0:HD], linv)
                # store on the ACT HWDGE ring: the SP ring gets congested by
                # the wave-0 chunk loads, which would delay wave 1
                w = 0 if i < 3 else 1
                nc.scalar.dma_start(
                    out=attn_cw[w][i if i < 3 else 0].rearrange(
                        "(h d) -> h d", d=HD),
                    in_=attn_sb)
                if i in (2, 3):
                    # overlap the attn AllGather wave with later users
                    nc.gpsimd.collective_compute(
                        "AllGather", mybir.AluOpType.bypass,
                        replica_groups=[list(range(NCORES))],
                        ins=[attn_cw[w].opt()], outs=[attn_agw[w].opt()])
                if i < 3:
                    _emit_wd_slab(2 * i)
                    _emit_wd_slab(2 * i + 1)

            # ---------------- phase D: dense output projection --------------
            # attnT column 4c + wave-user holds global user; built per wave so
            # wave 0 overlaps the last user's attention
            attnT_gs = [const.tile([128, KTG, U], WDT, name=f"attnT{g}",
                                   uniquify=True) for g in range(KT // KTG)]
            for w in range(2):
                nw = WAVE_USERS[w]
                attn_flat = attn_agw[w].rearrange("c j n -> (c j) n")
                for g6 in range(6):
                    wg = 768 if g6 < 5 else HID - 5 * 768
                    a_slab = upool.tile([NCORES * 3, 768], F32, tag="achunk",
                                        name="a_slab")
                    nc.sync.dma_start(
                        out=a_slab[0:NCORES * nw, 0:wg],
                        in_=attn_flat[:, g6 * 768:g6 * 768 + wg])
                    for tt in range(6):
                        t = 6 * g6 + tt
                        cw = 128 if t < 35 else 64
                        ps_t2 = pstpool.tile([128, NCORES * 3], F32,
                                             tag="pst", name="ps_t2")
                        nc.tensor.transpose(
                            ps_t2[0:cw, 0:NCORES * nw],
                            a_slab[0:NCORES * nw, tt * 128:tt * 128 + cw],
                            identity[0:NCORES * nw, 0:NCORES * nw])
                        dst = attnT_gs[t // KTG][0:cw, t % KTG, :].rearrange(
                            "p (c r) -> p c r", r=UPC)[:, :, 3 * w:3 * w + nw]
                        src_ = ps_t2[0:cw, 0:NCORES * nw].rearrange(
                            "p (c j) -> p c j", j=nw)
                        nc.vector.tensor_copy(out=dst, in_=src_)

            DC = DN // 4  # 142
            psD = pspool.tile([128, DC], F32, tag="bank", name="psD")

            def _dense_mms(t, lhs):
                for j in range(4):
                    nc.tensor.matmul(psD[32 * j:32 * j + 32, :], lhs,
                                     _dense_rhs(t)[..., DC * j:DC * (j + 1)],
                                     start=(t == 0), stop=(t == 35),
                                     tile_position=(0, 32 * j))

            rhs_of = {}

            def _dense_rhs(t):
                return rhs_of[t]

            for g in range(17):
                if g >= len(wd_slabs):
                    _emit_wd_slab(g)
                wdslab = wd_slabs[g]
                for t2 in range(2):
                    t = 2 * g + t2
                    rhs_of[t] = wdslab[:, t2, :]
                    _dense_mms(t, attnT_gs[t // KTG][:, t % KTG, :])
            # k-tiles 34 (full) and 35 (64 rows)
            wd34 = wdpool.tile([128, 2, DN], WDT, tag="w", name="wd34")
            nc.sync.dma_start(
                out=wd34[:, 0:1, :],
                in_=wd[34 * 128:35 * 128, :].rearrange("(t p) n -> p t n",
                                                       p=128))
            rhs_of[34] = wd34[:, 0, :]
            _dense_mms(34, attnT_gs[34 // KTG][:, 34 % KTG, :])
            wdlast = wpool.tile([64, DN], WDT, tag="wl", name="wdlast")
            nc.sync.dma_start(out=wdlast, in_=wd[ROWS_FULL:HID, :])
            rhs_of[35] = wdlast[:, :]
            _dense_mms(35, attnT_gs[35 // KTG][0:64, 35 % KTG, :])

            outD = const.tile([128, DC], F32)
            nc.vector.tensor_copy(out=outD, in_=psD[:, :])
            outc_ji = bass.AP(
                tensor=outc.ap().tensor, offset=0,
                ap=[[DC, 4], [DN, U], [1, DC]])
            nc.sync.dma_start(out=outc_ji, in_=outD)

    nc.compile()
    return nc


def _rot_mat(cos_u, sin_u):
    """M such that M @ x = x*cos + rotate_half(x)*sin, for one user."""
    m = np.zeros((HD, HD), np.float32)
    np.fill_diagonal(m, cos_u)
    half = HD // 2
    for r in range(half):
        m[r, r + half] += -sin_u[r]
        m[r + half, r] += sin_u[r + half]
    return m


def kernel(hidden_states, cos, sin, k_cache, v_cache, attn_masks, w_qkv,
           w_dense, trace=False):
    global _prog, LAST_RESULT
    if _prog is None:
        _prog = _build()

    hidden_states = np.asarray(hidden_states, np.float32)
    cos = np.asarray(cos, np.float32)
    sin = np.asarray(sin, np.float32)
    k_cache = np.asarray(k_cache, np.float32)
    v_cache = np.asarray(v_cache, np.float32)
    attn_masks = np.asarray(attn_masks, np.float32)
    w_qkv = np.asarray(w_qkv, np.float32)
    w_dense = np.asarray(w_dense, np.float32)

    hT = np.ascontiguousarray(hidden_states[0].T)            # [4544, 32]
    wqT = np.zeros((HID, NCORES * NCOL), np.float32)
    wqT[:, :w_qkv.shape[0]] = w_qkv.T
    wdT = np.ascontiguousarray(w_dense.T)                    # [4544, 4544]

    in_maps = []
    for c in range(NCORES):
        us = slice(UPC * c, UPC * (c + 1))
        k_u = np.moveaxis(k_cache[:, 0, us], 1, 0).reshape(UPC, S, HD)
        kT_u = np.transpose(k_u, (0, 2, 1))                  # [4, 64, 8192]
        kT_pack = np.concatenate(
            [kT_u[:, :, :S // 2], kT_u[:, :, S // 2:]], axis=1)
        v_u = np.moveaxis(v_cache[:, 0, us], 1, 0).reshape(UPC, S, HD)
        m_u = np.moveaxis(attn_masks[:, 0, us], 1, 0).reshape(UPC, NT, 128)
        muT = np.stack([
            _rot_mat(cos[0, u, 0], sin[0, u, 0]).T
            for u in range(UPC * c, UPC * (c + 1))
        ])                                                   # [4, 64, 64]
        in_maps.append({
            "hT": hT,
            "wq": np.ascontiguousarray(wqT[:, NCOL * c:NCOL * (c + 1)]),
            "wd": np.ascontiguousarray(wdT[:, DN * c:DN * (c + 1)]),
            "kTc": np.ascontiguousarray(kT_pack),
            "vc": np.ascontiguousarray(v_u),
            "mc": np.ascontiguousarray(m_u),
            "muT": np.ascontiguousarray(np.transpose(muT, (1, 0, 2))),
        })

    res = run_bass_kernel_spmd(_prog, in_maps, list(range(NCORES)),
                               trace=trace)
    LAST_RESULT = res
    out = np.concatenate([res.results[c]["outc"] for c in range(NCORES)],
                         axis=1)                             # [32, 4544]
    return out[None].astype(np.float32)



# revision 8
# speedup vs baseline: 1.7157x; 1.7157x over previous
"""Trainium2 Bass kernel for Falcon-7B MQA flash-decode attention block.

Geometry (hardcoded from the problem spec):
  hidden [1, 32, 4544], w_qkv [4672, 4544] (71 q heads + 1 k + 1 v, hd=64),
  kv cache [4, 1, 32, 2048, 64], masks [4, 1, 32, 2048], w_dense [4544, 4544].

Sharding across 8 NeuronCores:
  - users (32) are data-parallel, 4 per core: each core holds its users' KV.
  - w_qkv / w_dense are tensor-parallel column-split 8 ways; an AllToAll
    redistributes the fused QKV activations from column-shards to user-shards,
    one AllGather collects attention outputs for the dense matmul.
  - all matmul operands are bf16 (host-cast); PSUM accumulation stays fp32.
    bf16 is 4x faster on the PE per moving row and halves HBM traffic.
  - softmax uses the shift-invariant (max-free) formulation; the attention
    mask rides along as contraction row 64 of kT with q row 64 = 8.0, so the
    ACT exp's 1/8 scale returns exactly qk/8 + mask. No separate mask ops.
  - scores are packed 7 per PSUM bank at stride 71; exp batches 14 s-tiles
    (2 banks) per ACT op to amortize the ~185ns fixed ACT latency.
  - attnT for the dense matmul comes from a strided DMA load of the gathered
    attention (DRAM->SBUF transposed AP), not PE transposes.

Host-side prep is layout-only (transposes / packing / dtype casts).
"""

import sys

if "/opt/trn_rl_repo" not in sys.path:
    sys.path.insert(0, "/opt/trn_rl_repo")

import numpy as np

import concourse.bacc as bacc
import concourse.bass as bass
import concourse.mybir as mybir
import concourse.tile as tile
from concourse.bass_utils import run_bass_kernel_spmd
from concourse.masks import make_identity

F32 = mybir.dt.float32
BF16 = mybir.dt.bfloat16

NCORES = 8
U = 32          # users total
UPC = 4         # users per core
HID = 4544
NH = 71         # query heads
HD = 64
HPC = 10        # heads per core in the padded qkv column split (8*10*64 = 5120)
NCOL = HPC * HD         # 640 fused columns per core
DN = HID // NCORES      # 568 dense output columns per core
S = 8192                # total cached tokens per user (4 chunks x 2048)
NT = S // 128           # 64 s-tiles of 128
KT = 36                 # k-tiles over HID (zero-padded to 4608 rows)
WQS = 6                 # wq k-tiles per DMA slab (6 slabs of 6)
QC = NCOL // 4          # 160 fused columns per PSUM quadrant
DC = DN // 4            # 142 dense columns per PSUM quadrant
EG = (14, 14, 14, 14, 8)  # exp batch sizes over the 64 s-tiles

LAST_RESULT = None
_prog = None


def _build():
    nc = bacc.Bacc("TRN2", target_bir_lowering=False, debug=False,
                   num_devices=NCORES)

    # host-packed inputs (see kernel() below)
    hT = nc.dram_tensor("hT", [128, KT, U], BF16, kind="ExternalInput")
    wq = nc.dram_tensor("wq", [128, KT, NCOL], BF16, kind="ExternalInput")
    wd = nc.dram_tensor("wd", [128, KT, DN], BF16, kind="ExternalInput")
    kTc = nc.dram_tensor("kTc", [UPC, HD + 1, S], BF16, kind="ExternalInput")
    vc = nc.dram_tensor("vc", [UPC, 128, NT, HD + 1], BF16,
                        kind="ExternalInput")
    muT = nc.dram_tensor("muT", [HD, UPC, HD], F32, kind="ExternalInput")
    outc = nc.dram_tensor("outc", [U, DN], F32, kind="ExternalOutput")

    with tile.TileContext(nc) as tc:
        with (
            tc.tile_pool(name="const", bufs=1) as const,
            tc.tile_pool(name="wpool", bufs=2) as wpool,
            tc.tile_pool(name="kpool", bufs=2) as kpool,
            tc.tile_pool(name="vpool", bufs=2) as vpool,
            tc.tile_pool(name="ppool", bufs=2) as ppool,
            tc.tile_pool(name="upool", bufs=2) as upool,
            tc.tile_pool(name="pqpool", bufs=1, space="PSUM") as pqpool,
            tc.tile_pool(name="psc", bufs=2, space="PSUM") as pscpool,
            tc.tile_pool(name="pvpool", bufs=1, space="PSUM") as pvpool,
            tc.tile_pool(name="pstpool", bufs=2, space="PSUM") as pstpool,
            tc.tile_pool(name="dram", bufs=1, space="DRAM") as dram,
        ):
            identity = const.tile([128, 128], F32)
            make_identity(nc, identity)

            # ---------------- phase A: fused QKV projection ----------------
            hT_all = const.tile([128, KT, U], BF16)
            nc.sync.dma_start(out=hT_all, in_=hT[:, :, :])
            muT_sb = const.tile([HD, UPC, HD], F32)
            nc.scalar.dma_start(out=muT_sb, in_=muT[:, :, :])

            psQ = pqpool.tile([128, QC], F32, tag="bank", name="psQ")
            wslabs = []
            for g in range(WQS):
                wslab = wpool.tile([128, WQS, NCOL], BF16, tag="w",
                                   name="wslab", uniquify=True)
                if g == 0:
                    # split the first slab so the projection starts after two
                    # k-tiles instead of the full slab
                    nc.sync.dma_start(out=wslab[:, 0:2, :],
                                      in_=wq[:, 0:2, :])
                    nc.sync.dma_start(out=wslab[:, 2:WQS, :],
                                      in_=wq[:, 2:WQS, :])
                else:
                    nc.sync.dma_start(out=wslab,
                                      in_=wq[:, WQS * g:WQS * (g + 1), :])
                wslabs.append(wslab)
            for g in range(WQS):
                for t6 in range(WQS):
                    t = WQS * g + t6
                    lhs = hT_all[:, t, :]
                    for j in range(4):
                        nc.tensor.matmul(
                            psQ[32 * j:32 * j + 32, :], lhs,
                            wslabs[g][:, t6, QC * j:QC * (j + 1)],
                            start=(t == 0), stop=(t == KT - 1),
                            tile_position=(0, 32 * j))

            fq_sb = const.tile([128, QC], BF16)
            nc.scalar.activation(out=fq_sb, in_=psQ[:, :],
                                 func=mybir.ActivationFunctionType.Copy)

            fused_x = dram.tile([U, NCOL], BF16)
            fused_x_ji = bass.AP(
                tensor=fused_x.tensor, offset=fused_x.offset,
                ap=[[QC, 4], [NCOL, U], [1, QC]])
            # ACT-ring store: the SP ring is busy prefetching KV
            nc.scalar.dma_start(out=fused_x_ji, in_=fq_sb)
            # block d of the flat input (users 4d..4d+3) goes to core d
            fused_loc = dram.tile([NCORES, UPC, NCOL], BF16)
            nc.gpsimd.collective_compute(
                "AllToAll", mybir.AluOpType.bypass,
                replica_groups=[list(range(NCORES))],
                ins=[fused_x.opt()], outs=[fused_loc.opt()])

            # one strided gather for all 4 local users: (head, user, d)
            q_bf = const.tile([80, UPC, HD], BF16)
            for i in range(UPC):
                nc.scalar.dma_start(
                    out=q_bf[:, i, :],
                    in_=bass.AP(
                        tensor=fused_loc.tensor,
                        offset=fused_loc.offset + i * NCOL,
                        ap=[[UPC * NCOL, NCORES], [HD, HPC], [1, HD]]))
            q_f32 = const.tile([NH + 1, UPC, HD], F32)
            nc.vector.tensor_copy(out=q_f32, in_=q_bf[0:NH + 1, :, :])
            vcur_all = const.tile([1, UPC, HD + 1], BF16)  # [v_cur | 1]
            nc.scalar.dma_start(
                out=vcur_all[:, :, 0:HD],
                in_=fused_loc[7, :, 2 * HD:3 * HD][None, :, :])
            nc.vector.memset(vcur_all[:, :, HD:HD + 1], 1.0)

            # ---------------- phase C: per-user flash-decode attention ------
            HIDP = KT * 128  # attn padded to 4608 so xbar tiles divide
            attn_c = dram.tile([UPC, HIDP], BF16, name="attn_c")
            zero4 = const.tile([UPC, HD], BF16)
            nc.vector.memset(zero4, 0.0)
            nc.scalar.dma_start(
                out=bass.AP(tensor=attn_c.tensor,
                            offset=attn_c.offset + HID,
                            ap=[[HIDP, UPC], [1, HD]]),
                in_=zero4)
            attn_ag = dram.tile([NCORES, UPC, HIDP], BF16,
                                addr_space="Shared", name="attn_ag")

            for i in range(UPC):
                # [k^T | mask row]: contraction row 64 carries the mask; the
                # q side puts 8.0 there so exp's 1/8 scale yields qk/8 + m
                kT_sb = kpool.tile([HD + 1, S], BF16, tag="kT", name="kT_sb")
                nc.sync.dma_start(out=kT_sb, in_=kTc[i])
                vones = vpool.tile([128, NT, HD + 1], BF16, tag="v",
                                   name="vones")
                nc.sync.dma_start(out=vones, in_=vc[i])

                # q heads 0..70 plus the shared k head at row 71, transposed
                ps_qT = pstpool.tile([HD, NH + 1], F32, tag="pst",
                                     name="ps_qT")
                nc.tensor.transpose(ps_qT, q_f32[0:NH + 1, i, :],
                                    identity[0:NH + 1, 0:NH + 1])
                qkT = upool.tile([HD, NH + 1], F32, tag="qkT", name="qkT")
                nc.vector.tensor_copy(out=qkT, in_=ps_qT)

                # rotary as a matmul; row 64 = 8.0 scales the kT mask row
                ps_rot = pstpool.tile([HD, NH + 1], F32, tag="pst",
                                      name="ps_rot")
                nc.tensor.matmul(ps_rot, muT_sb[:, i, :], qkT,
                                 start=True, stop=True)
                qTr = upool.tile([HD + 1, NH + 1], BF16, tag="qTr",
                                 name="qTr")
                nc.vector.tensor_copy(out=qTr[0:HD, :], in_=ps_rot)
                nc.vector.memset(qTr[HD:HD + 1, :], 8.0)

                # current-token score for all heads: [1, 71] (no mask row)
                ps_sc = pstpool.tile([1, NH], F32, tag="pst", name="ps_sc")
                nc.tensor.matmul(ps_sc, qTr[0:HD, NH:NH + 1],
                                 qTr[0:HD, 0:NH], start=True, stop=True)
                curw = upool.tile([1, NH], BF16, tag="curw", name="curw")
                nc.scalar.activation(out=curw, in_=ps_sc,
                                     func=mybir.ActivationFunctionType.Exp,
                                     scale=0.125)

                # scores + exp over the 64 s-tiles, 7 tiles per PSUM bank at
                # stride 71, one batched exp per 2-bank group
                pT_all = ppool.tile([128, NT, NH], BF16, tag="pT",
                                    name="pT_all")
                pv = pvpool.tile([NH, HD + 1], F32, tag="pv", name="pv")
                t0 = 0
                for gi, gn in enumerate(EG):
                    ps_g = pscpool.tile([128, 2, 512], F32, tag="sg",
                                        name="ps_g")
                    for k in range(gn):
                        t = t0 + k
                        nc.tensor.matmul(
                            ps_g[:, k // 7, (k % 7) * NH:(k % 7 + 1) * NH],
                            kT_sb[:, t * 128:(t + 1) * 128],
                            qTr[:, 0:NH], start=True, stop=True)
                    nb = (gn + 6) // 7
                    for b in range(nb):
                        bn = min(7, gn - 7 * b)
                        nc.scalar.activation(
                            out=pT_all[:, t0 + 7 * b:t0 + 7 * b + bn, :],
                            in_=ps_g[:, b, 0:bn * NH].rearrange(
                                "p (k h) -> p k h", h=NH),
                            func=mybir.ActivationFunctionType.Exp,
                            scale=0.125)
                    t0 += gn

                # PV with fused row-sum via the ones column
                for t in range(NT):
                    nc.tensor.matmul(pv, pT_all[:, t, :], vones[:, t, :],
                                     start=(t == 0), stop=False)
                nc.tensor.matmul(pv, curw, vcur_all[:, i, :], start=False,
                                 stop=True)

                linv = upool.tile([NH, 1], F32, tag="linv", name="linv")
                nc.vector.reciprocal(out=linv, in_=pv[:, HD:HD + 1])
                attn_sb = upool.tile([NH, HD], BF16, tag="attn",
                                     name="attn_sb")
                nc.vector.tensor_scalar_mul(attn_sb, pv[:, 0:HD], linv)
                nc.scalar.dma_start(
                    out=bass.AP(tensor=attn_c.tensor,
                                offset=attn_c.offset + i * HIDP,
                                ap=[[HD, NH], [1, HD]]),
                    in_=attn_sb)

            nc.gpsimd.collective_compute(
                "AllGather", mybir.AluOpType.bypass,
                replica_groups=[list(range(NCORES))],
                ins=[attn_c.opt()], outs=[attn_ag.opt()])

            # ---------------- phase D: dense output projection --------------
            # dense weights prefetch during attention (4 slabs of 9 k-tiles)
            wd_sb = const.tile([128, KT, DN], BF16)
            for g in range(4):
                nc.sync.dma_start(out=wd_sb[:, 9 * g:9 * (g + 1), :],
                                  in_=wd[:, 9 * g:9 * (g + 1), :])

            # attnT via one xbar DMA transpose of the gathered activations
            attnT = const.tile([128, KT, U], BF16)
            nc.sync.dma_start_transpose(
                out=attnT, in_=attn_ag.rearrange("c j n -> (c j) n"))

            psD = pqpool.tile([128, DC], F32, tag="bank", name="psD")
            for t in range(KT):
                for j in range(4):
                    nc.tensor.matmul(psD[32 * j:32 * j + 32, :],
                                     attnT[:, t, :],
                                     wd_sb[:, t, DC * j:DC * (j + 1)],
                                     start=(t == 0), stop=(t == KT - 1),
                                     tile_position=(0, 32 * j))

            outD = const.tile([128, DC], F32)
            nc.vector.tensor_copy(out=outD, in_=psD[:, :])
            outc_ji = bass.AP(
                tensor=outc.ap().tensor, offset=0,
                ap=[[DC, 4], [DN, U], [1, DC]])
            nc.scalar.dma_start(out=outc_ji, in_=outD)

    nc.compile()
    return nc


def _rot_mat(cos_u, sin_u):
    """M such that M @ x = x*cos + rotate_half(x)*sin, for one user."""
    m = np.zeros((HD, HD), np.float32)
    np.fill_diagonal(m, cos_u)
    half = HD // 2
    for r in range(half):
        m[r, r + half] += -sin_u[r]
        m[r + half, r] += sin_u[r + half]
    return m


def kernel(hidden_states, cos, sin, k_cache, v_cache, attn_masks, w_qkv,
           w_dense, trace=False):
    global _prog, LAST_RESULT
    import ml_dtypes

    bf16 = ml_dtypes.bfloat16
    if _prog is None:
        _prog = _build()

    hidden_states = np.asarray(hidden_states, np.float32)
    cos = np.asarray(cos, np.float32)
    sin = np.asarray(sin, np.float32)
    k_cache = np.asarray(k_cache, np.float32)
    v_cache = np.asarray(v_cache, np.float32)
    attn_masks = np.asarray(attn_masks, np.float32)
    w_qkv = np.asarray(w_qkv, np.float32)
    w_dense = np.asarray(w_dense, np.float32)

    def pack_k(m, ncol):
        """[4544, ncol] -> [128, 36, ncol] bf16, zero-padded to 4608 rows."""
        p = np.zeros((KT * 128, ncol), np.float32)
        p[:m.shape[0]] = m
        return np.ascontiguousarray(
            p.reshape(KT, 128, ncol).transpose(1, 0, 2).astype(bf16))

    hT = pack_k(hidden_states[0].T, U)                       # [128, 36, 32]
    wqT = np.zeros((HID, NCORES * NCOL), np.float32)
    wqT[:, :w_qkv.shape[0]] = w_qkv.T
    wdT = w_dense.T                                          # [4544, 4544]

    in_maps = []
    for c in range(NCORES):
        us = slice(UPC * c, UPC * (c + 1))
        k_u = np.moveaxis(k_cache[:, 0, us], 1, 0).reshape(UPC, S, HD)
        m_u = np.moveaxis(attn_masks[:, 0, us], 1, 0).reshape(UPC, S)
        kT_u = np.concatenate(
            [np.transpose(k_u, (0, 2, 1)), m_u[:, None, :]], axis=1)
        v_u = np.moveaxis(v_cache[:, 0, us], 1, 0).reshape(UPC, NT, 128, HD)
        vones = np.concatenate(
            [v_u, np.ones((UPC, NT, 128, 1), np.float32)], axis=3)
        muT = np.stack([
            _rot_mat(cos[0, u, 0], sin[0, u, 0]).T
            for u in range(UPC * c, UPC * (c + 1))
        ])                                                   # [4, 64, 64]
        in_maps.append({
            "hT": hT,
            "wq": pack_k(wqT[:, NCOL * c:NCOL * (c + 1)], NCOL),
            "wd": pack_k(wdT[:, DN * c:DN * (c + 1)], DN),
            "kTc": np.ascontiguousarray(kT_u.astype(bf16)),
            "vc": np.ascontiguousarray(
                vones.transpose(0, 2, 1, 3).astype(bf16)),
            "muT": np.ascontiguousarray(
                np.transpose(muT, (1, 0, 2)).astype(np.float32)),
        })

    res = run_bass_kernel_spmd(_prog, in_maps, list(range(NCORES)),
                               trace=trace)
    LAST_RESULT = res
    out = np.concatenate([res.results[c]["outc"] for c in range(NCORES)],
                         axis=1)                             # [32, 4544]
    return out[None].astype(np.float32)


# revision 9
# speedup vs baseline: 1.9925x; 1.1614x over previous
"""Trainium2 Bass kernel for Falcon-7B MQA flash-decode attention block.

Geometry (hardcoded from the problem spec):
  hidden [1, 32, 4544], w_qkv [4672, 4544] (71 q heads + 1 k + 1 v, hd=64),
  kv cache [4, 1, 32, 2048, 64], masks [4, 1, 32, 2048], w_dense [4544, 4544].

Sharding across 8 NeuronCores:
  - users (32) are data-parallel, 4 per core: each core holds its users' KV.
  - w_qkv / w_dense are tensor-parallel column-split 8 ways; an AllToAll
    redistributes the fused QKV activations from column-shards to user-shards,
    one AllGather collects attention outputs for the dense matmul.
  - all matmul operands are bf16 (host-cast); PSUM accumulation stays fp32.
    bf16 is 4x faster on the PE per moving row and halves HBM traffic.
  - softmax uses the shift-invariant (max-free) formulation; the attention
    mask rides along as contraction row 64 of kT with q row 64 = 8.0, so the
    ACT exp's 1/8 scale returns exactly qk/8 + mask. No separate mask ops.
  - scores are packed 7 per PSUM bank at stride 71; exp batches 14 s-tiles
    (2 banks) per ACT op to amortize the ~185ns fixed ACT latency.
  - attnT for the dense matmul comes from a strided DMA load of the gathered
    attention (DRAM->SBUF transposed AP), not PE transposes.

Host-side prep is layout-only (transposes / packing / dtype casts).
"""

import sys

if "/opt/trn_rl_repo" not in sys.path:
    sys.path.insert(0, "/opt/trn_rl_repo")

import numpy as np

import concourse.bacc as bacc
import concourse.bass as bass
import concourse.mybir as mybir
import concourse.tile as tile
from concourse.bass_utils import run_bass_kernel_spmd
from concourse.masks import make_identity

F32 = mybir.dt.float32
BF16 = mybir.dt.bfloat16

NCORES = 8
U = 32          # users total
UPC = 4         # users per core
HID = 4544
NH = 71         # query heads
HD = 64
HPC = 10        # heads per core in the padded qkv column split (8*10*64 = 5120)
NCOL = HPC * HD         # 640 fused columns per core
DN = HID // NCORES      # 568 dense output columns per core
S = 8192                # total cached tokens per user (4 chunks x 2048)
NT = S // 128           # 64 s-tiles of 128
KT = 36                 # k-tiles over HID (zero-padded to 4608 rows)
WQS = 6                 # wq k-tiles per DMA slab (6 slabs of 6)
QC = NCOL // 4          # 160 fused columns per PSUM quadrant
DC = DN // 4            # 142 dense columns per PSUM quadrant
EG = (14, 14, 14, 14, 8)  # exp batch sizes over the 64 s-tiles

LAST_RESULT = None
_prog = None


def _build():
    nc = bacc.Bacc("TRN2", target_bir_lowering=False, debug=False,
                   num_devices=NCORES)

    # host-packed inputs (see kernel() below)
    hT = nc.dram_tensor("hT", [128, KT, U], BF16, kind="ExternalInput")
    wq = nc.dram_tensor("wq", [128, KT, NCOL], BF16, kind="ExternalInput")
    wd = nc.dram_tensor("wd", [128, KT, DN], BF16, kind="ExternalInput")
    kTc = nc.dram_tensor("kTc", [UPC, HD + 1, S], BF16, kind="ExternalInput")
    vc = nc.dram_tensor("vc", [UPC, 128, NT, HD + 1], BF16,
                        kind="ExternalInput")
    muT = nc.dram_tensor("muT", [HD, UPC, HD], F32, kind="ExternalInput")
    outc = nc.dram_tensor("outc", [U, DN], F32, kind="ExternalOutput")

    with tile.TileContext(nc) as tc:
        with (
            tc.tile_pool(name="const", bufs=1) as const,
            tc.tile_pool(name="wpool", bufs=2) as wpool,
            tc.tile_pool(name="kpool", bufs=2) as kpool,
            tc.tile_pool(name="vpool", bufs=2) as vpool,
            tc.tile_pool(name="ppool", bufs=2) as ppool,
            tc.tile_pool(name="upool", bufs=2) as upool,
            tc.tile_pool(name="pqpool", bufs=1, space="PSUM") as pqpool,
            tc.tile_pool(name="psc", bufs=2, space="PSUM") as pscpool,
            tc.tile_pool(name="pvpool", bufs=1, space="PSUM") as pvpool,
            tc.tile_pool(name="pstpool", bufs=2, space="PSUM") as pstpool,
            tc.tile_pool(name="dram", bufs=1, space="DRAM") as dram,
        ):
            identity = const.tile([128, 128], F32)
            make_identity(nc, identity)

            # warm the PE p-state during the initial weight-DMA wait: ~3us of
            # continuous dummy matmuls take the clock 0.65 -> 2.4 GHz before
            # the first real QKV matmul lands
            wtile = const.tile([128, 128], BF16)
            nc.vector.memset(wtile, 0.0)
            ps_w = pscpool.tile([128, 2, 512], F32, tag="sg", name="ps_w")
            for w in range(30):
                nc.tensor.matmul(ps_w[0:1, 0, 0:128], wtile[:, 0:1],
                                 wtile[:, 0:128], start=True, stop=True)

            # ---------------- phase A: fused QKV projection ----------------
            hT_all = const.tile([128, KT, U], BF16)
            nc.sync.dma_start(out=hT_all, in_=hT[:, :, :])
            muT_sb = const.tile([HD, UPC, HD], F32)
            nc.scalar.dma_start(out=muT_sb, in_=muT[:, :, :])

            psQ = pqpool.tile([128, QC], F32, tag="bank", name="psQ")
            wslabs = []
            for g in range(WQS):
                wslab = wpool.tile([128, WQS, NCOL], BF16, tag="w",
                                   name="wslab", uniquify=True)
                if g == 0:
                    # split the first slab so the projection starts after two
                    # k-tiles instead of the full slab
                    nc.sync.dma_start(out=wslab[:, 0:2, :],
                                      in_=wq[:, 0:2, :])
                    nc.sync.dma_start(out=wslab[:, 2:WQS, :],
                                      in_=wq[:, 2:WQS, :])
                else:
                    nc.sync.dma_start(out=wslab,
                                      in_=wq[:, WQS * g:WQS * (g + 1), :])
                wslabs.append(wslab)
            for g in range(WQS):
                for t6 in range(WQS):
                    t = WQS * g + t6
                    lhs = hT_all[:, t, :]
                    for j in range(4):
                        nc.tensor.matmul(
                            psQ[32 * j:32 * j + 32, :], lhs,
                            wslabs[g][:, t6, QC * j:QC * (j + 1)],
                            start=(t == 0), stop=(t == KT - 1),
                            tile_position=(0, 32 * j))

            fq_sb = const.tile([128, QC], BF16)
            nc.scalar.activation(out=fq_sb, in_=psQ[:, :],
                                 func=mybir.ActivationFunctionType.Copy)

            fused_x = dram.tile([U, NCOL], BF16)
            fused_x_ji = bass.AP(
                tensor=fused_x.tensor, offset=fused_x.offset,
                ap=[[QC, 4], [NCOL, U], [1, QC]])
            # ACT-ring store: the SP ring is busy prefetching KV
            nc.scalar.dma_start(out=fused_x_ji, in_=fq_sb)
            # block d of the flat input (users 4d..4d+3) goes to core d
            fused_loc = dram.tile([NCORES, UPC, NCOL], BF16)
            nc.gpsimd.collective_compute(
                "AllToAll", mybir.AluOpType.bypass,
                replica_groups=[list(range(NCORES))],
                ins=[fused_x.opt()], outs=[fused_loc.opt()])

            # one strided gather for all 4 local users: (head, user, d)
            q_bf = const.tile([80, UPC, HD], BF16)
            for i in range(UPC):
                nc.scalar.dma_start(
                    out=q_bf[:, i, :],
                    in_=bass.AP(
                        tensor=fused_loc.tensor,
                        offset=fused_loc.offset + i * NCOL,
                        ap=[[UPC * NCOL, NCORES], [HD, HPC], [1, HD]]))
            q_f32 = const.tile([NH + 1, UPC, HD], F32)
            for i in range(UPC):
                nc.vector.tensor_copy(out=q_f32[:, i, :],
                                      in_=q_bf[0:NH + 1, i, :])
            vcur_all = const.tile([1, UPC, HD + 1], BF16)  # [v_cur | 1]
            nc.scalar.dma_start(
                out=vcur_all[:, :, 0:HD],
                in_=fused_loc[7, :, 2 * HD:3 * HD][None, :, :])
            nc.vector.memset(vcur_all[:, :, HD:HD + 1], 1.0)

            wd_sb = const.tile([128, KT, DN], BF16)

            # ---------------- phase C: per-user flash-decode attention ------
            HIDP = KT * 128  # attn padded to 4608 so xbar tiles divide
            attn_c = dram.tile([UPC, HIDP], BF16, name="attn_c")
            zero4 = const.tile([UPC, HD], BF16)
            nc.vector.memset(zero4, 0.0)
            nc.scalar.dma_start(
                out=bass.AP(tensor=attn_c.tensor,
                            offset=attn_c.offset + HID,
                            ap=[[HIDP, UPC], [1, HD]]),
                in_=zero4)
            attn_ag = dram.tile([NCORES, UPC, HIDP], BF16,
                                addr_space="Shared", name="attn_ag")

            for i in range(UPC):
                # [k^T | mask row]: contraction row 64 carries the mask; the
                # q side puts 8.0 there so exp's 1/8 scale yields qk/8 + m
                kT_sb = kpool.tile([HD + 1, S], BF16, tag="kT", name="kT_sb")
                vones = vpool.tile([128, NT, HD + 1], BF16, tag="v",
                                   name="vones")
                if i < 2:
                    # guard: users 0/1 KV loads start only once phase A's
                    # weight traffic is done (fq_sb written), keeping the wq
                    # slabs sole owners of the DMA engines
                    nc.vector.tensor_copy(out=kT_sb[0:1, 0:1],
                                          in_=fq_sb[0:1, 0:1])
                    nc.vector.tensor_copy(out=vones[0:1, 0:1, 0:1],
                                          in_=fq_sb[0:1, 0:1])
                nc.sync.dma_start(out=kT_sb, in_=kTc[i])
                nc.sync.dma_start(out=vones, in_=vc[i])
                if i == 0:
                    # guard: dense weights prefetch behind user-0 KV, i.e.
                    # after the fused_x store has claimed the DMA engines
                    for g in range(4):
                        nc.vector.tensor_copy(
                            out=wd_sb[0:1, 9 * g:9 * g + 1, 0:1],
                            in_=vones[0:1, 0:1, 0:1])
                    for g in range(4):
                        nc.sync.dma_start(
                            out=wd_sb[:, 9 * g:9 * (g + 1), :],
                            in_=wd[:, 9 * g:9 * (g + 1), :])

                # q heads 0..70 plus the shared k head at row 71, transposed
                ps_qT = pstpool.tile([HD, NH + 1], F32, tag="pst",
                                     name="ps_qT")
                nc.tensor.transpose(ps_qT, q_f32[0:NH + 1, i, :],
                                    identity[0:NH + 1, 0:NH + 1])
                qkT = upool.tile([HD, NH + 1], F32, tag="qkT", name="qkT")
                nc.vector.tensor_copy(out=qkT, in_=ps_qT)

                # rotary as a matmul; row 64 = 8.0 scales the kT mask row
                ps_rot = pstpool.tile([HD, NH + 1], F32, tag="pst",
                                      name="ps_rot")
                nc.tensor.matmul(ps_rot, muT_sb[:, i, :], qkT,
                                 start=True, stop=True)
                qTr = upool.tile([HD + 1, NH + 1], BF16, tag="qTr",
                                 name="qTr")
                nc.vector.tensor_copy(out=qTr[0:HD, :], in_=ps_rot)
                nc.vector.memset(qTr[HD:HD + 1, :], 8.0)

                # current-token score for all heads: [1, 71] (no mask row)
                ps_sc = pstpool.tile([1, NH], F32, tag="pst", name="ps_sc")
                nc.tensor.matmul(ps_sc, qTr[0:HD, NH:NH + 1],
                                 qTr[0:HD, 0:NH], start=True, stop=True)
                curw = upool.tile([1, NH], BF16, tag="curw", name="curw")
                nc.scalar.activation(out=curw, in_=ps_sc,
                                     func=mybir.ActivationFunctionType.Exp,
                                     scale=0.125)

                # scores + exp over the 64 s-tiles, 7 tiles per PSUM bank at
                # stride 71, one batched exp per 2-bank group
                pT_all = ppool.tile([128, NT, NH], BF16, tag="pT",
                                    name="pT_all")
                pv = pvpool.tile([NH, HD + 1], F32, tag="pv", name="pv")
                t0 = 0
                for gi, gn in enumerate(EG):
                    ps_g = pscpool.tile([128, 2, 512], F32, tag="sg",
                                        name="ps_g")
                    for k in range(gn):
                        t = t0 + k
                        nc.tensor.matmul(
                            ps_g[:, k // 7, (k % 7) * NH:(k % 7 + 1) * NH],
                            kT_sb[:, t * 128:(t + 1) * 128],
                            qTr[:, 0:NH], start=True, stop=True)
                    if gn == 14:
                        nc.scalar.activation(
                            out=pT_all[:, t0:t0 + 14, :],
                            in_=ps_g[:, :, 0:7 * NH].rearrange(
                                "p b (k h) -> p b k h", h=NH),
                            func=mybir.ActivationFunctionType.Exp,
                            scale=0.125)
                    else:
                        for b in range((gn + 6) // 7):
                            bn = min(7, gn - 7 * b)
                            nc.scalar.activation(
                                out=pT_all[:, t0 + 7 * b:t0 + 7 * b + bn, :],
                                in_=ps_g[:, b, 0:bn * NH].rearrange(
                                    "p (k h) -> p k h", h=NH),
                                func=mybir.ActivationFunctionType.Exp,
                                scale=0.125)
                    t0 += gn

                # PV with fused row-sum via the ones column
                for t in range(NT):
                    nc.tensor.matmul(pv, pT_all[:, t, :], vones[:, t, :],
                                     start=(t == 0), stop=False)
                nc.tensor.matmul(pv, curw, vcur_all[:, i, :], start=False,
                                 stop=True)

                linv = upool.tile([NH, 1], F32, tag="linv", name="linv")
                nc.vector.reciprocal(out=linv, in_=pv[:, HD:HD + 1])
                attn_sb = upool.tile([NH, HD], BF16, tag="attn",
                                     name="attn_sb")
                nc.vector.tensor_scalar_mul(attn_sb, pv[:, 0:HD], linv)
                nc.scalar.dma_start(
                    out=bass.AP(tensor=attn_c.tensor,
                                offset=attn_c.offset + i * HIDP,
                                ap=[[HD, NH], [1, HD]]),
                    in_=attn_sb)

            nc.gpsimd.collective_compute(
                "AllGather", mybir.AluOpType.bypass,
                replica_groups=[list(range(NCORES))],
                ins=[attn_c.opt()], outs=[attn_ag.opt()])

            # ---------------- phase D: dense output projection --------------
            # attnT via one xbar DMA transpose of the gathered activations
            attnT = const.tile([128, KT, U], BF16)
            nc.sync.dma_start_transpose(
                out=attnT, in_=attn_ag.rearrange("c j n -> (c j) n"))

            psD = pqpool.tile([128, DC], F32, tag="bank", name="psD")
            for t in range(KT):
                for j in range(4):
                    nc.tensor.matmul(psD[32 * j:32 * j + 32, :],
                                     attnT[:, t, :],
                                     wd_sb[:, t, DC * j:DC * (j + 1)],
                                     start=(t == 0), stop=(t == KT - 1),
                                     tile_position=(0, 32 * j))

            outD = const.tile([128, DC], F32)
            nc.vector.tensor_copy(out=outD, in_=psD[:, :])
            outc_ji = bass.AP(
                tensor=outc.ap().tensor, offset=0,
                ap=[[DC, 4], [DN, U], [1, DC]])
            nc.scalar.dma_start(out=outc_ji, in_=outD)

    nc.compile()
    return nc


def _rot_mat(cos_u, sin_u):
    """M such that M @ x = x*cos + rotate_half(x)*sin, for one user."""
    m = np.zeros((HD, HD), np.float32)
    np.fill_diagonal(m, cos_u)
    half = HD // 2
    for r in range(half):
        m[r, r + half] += -sin_u[r]
        m[r + half, r] += sin_u[r + half]
    return m


def kernel(hidden_states, cos, sin, k_cache, v_cache, attn_masks, w_qkv,
           w_dense, trace=False):
    global _prog, LAST_RESULT
    import ml_dtypes

    bf16 = ml_dtypes.bfloat16
    if _prog is None:
        _prog = _build()

    hidden_states = np.asarray(hidden_states, np.float32)
    cos = np.asarray(cos, np.float32)
    sin = np.asarray(sin, np.float32)
    k_cache = np.asarray(k_cache, np.float32)
    v_cache = np.asarray(v_cache, np.float32)
    attn_masks = np.asarray(attn_masks, np.float32)
    w_qkv = np.asarray(w_qkv, np.float32)
    w_dense = np.asarray(w_dense, np.float32)

    def pack_k(m, ncol):
        """[4544, ncol] -> [128, 36, ncol] bf16, zero-padded to 4608 rows."""
        p = np.zeros((KT * 128, ncol), np.float32)
        p[:m.shape[0]] = m
        return np.ascontiguousarray(
            p.reshape(KT, 128, ncol).transpose(1, 0, 2).astype(bf16))

    hT = pack_k(hidden_states[0].T, U)                       # [128, 36, 32]
    wqT = np.zeros((HID, NCORES * NCOL), np.float32)
    wqT[:, :w_qkv.shape[0]] = w_qkv.T
    wdT = w_dense.T                                          # [4544, 4544]

    in_maps = []
    for c in range(NCORES):
        us = slice(UPC * c, UPC * (c + 1))
        k_u = np.moveaxis(k_cache[:, 0, us], 1, 0).reshape(UPC, S, HD)
        m_u = np.moveaxis(attn_masks[:, 0, us], 1, 0).reshape(UPC, S)
        kT_u = np.concatenate(
            [np.transpose(k_u, (0, 2, 1)), m_u[:, None, :]], axis=1)
        v_u = np.moveaxis(v_cache[:, 0, us], 1, 0).reshape(UPC, NT, 128, HD)
        vones = np.concatenate(
            [v_u, np.ones((UPC, NT, 128, 1), np.float32)], axis=3)
        muT = np.stack([
            _rot_mat(cos[0, u, 0], sin[0, u, 0]).T
            for u in range(UPC * c, UPC * (c + 1))
        ])                                                   # [4, 64, 64]
        in_maps.append({
            "hT": hT,
            "wq": pack_k(wqT[:, NCOL * c:NCOL * (c + 1)], NCOL),
            "wd": pack_k(wdT[:, DN * c:DN * (c + 1)], DN),
            "kTc": np.ascontiguousarray(kT_u.astype(bf16)),
            "vc": np.ascontiguousarray(
                vones.transpose(0, 2, 1, 3).astype(bf16)),
            "muT": np.ascontiguousarray(
                np.transpose(muT, (1, 0, 2)).astype(np.float32)),
        })

    res = run_bass_kernel_spmd(_prog, in_maps, list(range(NCORES)),
                               trace=trace)
    LAST_RESULT = res
    out = np.concatenate([res.results[c]["outc"] for c in range(NCORES)],
                         axis=1)                             # [32, 4544]
    return out[None].astype(np.float32)


# revision 11
# speedup vs baseline: 2.0220x; 1.0148x over previous
"""Trainium2 Bass kernel for Falcon-7B MQA flash-decode attention block.

Geometry (hardcoded from the problem spec):
  hidden [1, 32, 4544], w_qkv [4672, 4544] (71 q heads + 1 k + 1 v, hd=64),
  kv cache [4, 1, 32, 2048, 64], masks [4, 1, 32, 2048], w_dense [4544, 4544].

Sharding across 8 NeuronCores:
  - users (32) are data-parallel, 4 per core: each core holds its users' KV.
  - w_qkv / w_dense are tensor-parallel column-split 8 ways; an AllToAll
    redistributes the fused QKV activations from column-shards to user-shards,
    one AllGather collects attention outputs for the dense matmul.
  - all matmul operands are bf16 (host-cast); PSUM accumulation stays fp32.
    bf16 is 4x faster on the PE per moving row and halves HBM traffic.
  - softmax uses the shift-invariant (max-free) formulation; the attention
    mask rides along as contraction row 64 of kT with q row 64 = 8.0, so the
    ACT exp's 1/8 scale returns exactly qk/8 + mask. No separate mask ops.
  - scores are packed 7 per PSUM bank at stride 71; exp batches 14 s-tiles
    (2 banks) per ACT op to amortize the ~185ns fixed ACT latency.
  - attnT for the dense matmul comes from a strided DMA load of the gathered
    attention (DRAM->SBUF transposed AP), not PE transposes.

Host-side prep is layout-only (transposes / packing / dtype casts).
"""

import sys

if "/opt/trn_rl_repo" not in sys.path:
    sys.path.insert(0, "/opt/trn_rl_repo")

import numpy as np

import concourse.bacc as bacc
import concourse.bass as bass
import concourse.mybir as mybir
import concourse.tile as tile
from concourse.bass_utils import run_bass_kernel_spmd
from concourse.masks import make_identity

F32 = mybir.dt.float32
BF16 = mybir.dt.bfloat16

NCORES = 8
U = 32          # users total
UPC = 4         # users per core
HID = 4544
NH = 71         # query heads
HD = 64
HPC = 10        # heads per core in the padded qkv column split (8*10*64 = 5120)
NCOL = HPC * HD         # 640 fused columns per core
DN = HID // NCORES      # 568 dense output columns per core
S = 8192                # total cached tokens per user (4 chunks x 2048)
NT = S // 128           # 64 s-tiles of 128
KT = 36                 # k-tiles over HID (zero-padded to 4608 rows)
WQS = 6                 # wq k-tiles per DMA slab (6 slabs of 6)
QC = NCOL // 4          # 160 fused columns per PSUM quadrant
DC = DN // 4            # 142 dense columns per PSUM quadrant
EG = (14, 14, 14, 14, 8)  # exp batch sizes over the 64 s-tiles

LAST_RESULT = None
_prog = None


def _build():
    nc = bacc.Bacc("TRN2", target_bir_lowering=False, debug=False,
                   num_devices=NCORES)

    # host-packed inputs (see kernel() below)
    hT = nc.dram_tensor("hT", [128, KT, U], BF16, kind="ExternalInput")
    wq = nc.dram_tensor("wq", [128, KT, NCOL], BF16, kind="ExternalInput")
    wd = nc.dram_tensor("wd", [128, KT, DN], BF16, kind="ExternalInput")
    kTc = nc.dram_tensor("kTc", [UPC, HD + 1, S], BF16, kind="ExternalInput")
    vc = nc.dram_tensor("vc", [UPC, 128, NT, HD + 1], BF16,
                        kind="ExternalInput")
    muT = nc.dram_tensor("muT", [HD, UPC, HD], F32, kind="ExternalInput")
    outc = nc.dram_tensor("outc", [U, DN], F32, kind="ExternalOutput")

    with tile.TileContext(nc) as tc:
        with (
            tc.tile_pool(name="const", bufs=1) as const,
            tc.tile_pool(name="wpool", bufs=3) as wpool,
            tc.tile_pool(name="kpool", bufs=2) as kpool,
            tc.tile_pool(name="vpool", bufs=2) as vpool,
            tc.tile_pool(name="ppool", bufs=2) as ppool,
            tc.tile_pool(name="upool", bufs=2) as upool,
            tc.tile_pool(name="pqpool", bufs=1, space="PSUM") as pqpool,
            tc.tile_pool(name="psc", bufs=2, space="PSUM") as pscpool,
            tc.tile_pool(name="pvpool", bufs=1, space="PSUM") as pvpool,
            tc.tile_pool(name="pstpool", bufs=2, space="PSUM") as pstpool,
            tc.tile_pool(name="dram", bufs=1, space="DRAM") as dram,
        ):
            identity = const.tile([128, 128], F32)
            make_identity(nc, identity)

            # warm the PE p-state during the initial weight-DMA wait: ~3us of
            # continuous dummy matmuls take the clock 0.65 -> 2.4 GHz before
            # the first real QKV matmul lands
            wtile = const.tile([128, 128], BF16)
            nc.vector.memset(wtile, 0.0)
            ps_w = pscpool.tile([128, 2, 512], F32, tag="sg", name="ps_w")
            for w in range(30):
                nc.tensor.matmul(ps_w[0:1, 0, 0:128], wtile[:, 0:1],
                                 wtile[:, 0:128], start=True, stop=True)

            # ---------------- phase A: fused QKV projection ----------------
            hT_all = const.tile([128, KT, U], BF16)
            nc.sync.dma_start(out=hT_all, in_=hT[:, :, :])
            muT_sb = const.tile([HD, UPC, HD], F32)
            nc.scalar.dma_start(out=muT_sb, in_=muT[:, :, :])

            psQ = pqpool.tile([128, QC], F32, tag="bank", name="psQ")
            wslabs = []
            for g in range(WQS):
                wslab = wpool.tile([128, WQS, NCOL], BF16, tag="w",
                                   name="wslab", uniquify=True)
                if g == 0:
                    # split the first slab so the projection starts after two
                    # k-tiles instead of the full slab
                    nc.sync.dma_start(out=wslab[:, 0:2, :],
                                      in_=wq[:, 0:2, :])
                    nc.sync.dma_start(out=wslab[:, 2:WQS, :],
                                      in_=wq[:, 2:WQS, :])
                else:
                    nc.sync.dma_start(out=wslab,
                                      in_=wq[:, WQS * g:WQS * (g + 1), :])
                wslabs.append(wslab)
            for g in range(WQS):
                for t6 in range(WQS):
                    t = WQS * g + t6
                    lhs = hT_all[:, t, :]
                    for j in range(4):
                        nc.tensor.matmul(
                            psQ[32 * j:32 * j + 32, :], lhs,
                            wslabs[g][:, t6, QC * j:QC * (j + 1)],
                            start=(t == 0), stop=(t == KT - 1),
                            tile_position=(0, 32 * j))

            fq_sb = const.tile([128, QC], BF16)
            nc.scalar.activation(out=fq_sb, in_=psQ[:, :],
                                 func=mybir.ActivationFunctionType.Copy)

            fused_x = dram.tile([U, NCOL], BF16)
            fused_x_ji = bass.AP(
                tensor=fused_x.tensor, offset=fused_x.offset,
                ap=[[QC, 4], [NCOL, U], [1, QC]])
            # ACT-ring store: the SP ring is busy prefetching KV
            nc.scalar.dma_start(out=fused_x_ji, in_=fq_sb)
            # block d of the flat input (users 4d..4d+3) goes to core d
            fused_loc = dram.tile([NCORES, UPC, NCOL], BF16)
            nc.gpsimd.collective_compute(
                "AllToAll", mybir.AluOpType.bypass,
                replica_groups=[list(range(NCORES))],
                ins=[fused_x.opt()], outs=[fused_loc.opt()])

            # one strided gather for all 4 local users: (head, user, d)
            q_bf = const.tile([80, UPC, HD], BF16)
            for i in range(UPC):
                nc.scalar.dma_start(
                    out=q_bf[:, i, :],
                    in_=bass.AP(
                        tensor=fused_loc.tensor,
                        offset=fused_loc.offset + i * NCOL,
                        ap=[[UPC * NCOL, NCORES], [HD, HPC], [1, HD]]))
            q_f32 = const.tile([NH + 1, UPC, HD], F32)
            for i in range(UPC):
                nc.vector.tensor_copy(out=q_f32[:, i, :],
                                      in_=q_bf[0:NH + 1, i, :])
            wd_sb = const.tile([128, KT, DN], BF16)
            # dense-weight prefetch: gated behind the last q gather so the
            # tiny post-A2A gathers are not stuck behind 14.5us of wd DMA
            for g in range(4):
                nc.vector.tensor_copy(
                    out=wd_sb[0:1, 9 * g:9 * g + 1, 0:1],
                    in_=q_bf[0:1, UPC - 1, 0:1])
            for g in range(4):
                nc.sync.dma_start(
                    out=wd_sb[:, 9 * g:9 * (g + 1), :],
                    in_=wd[:, 9 * g:9 * (g + 1), :])
            vcur_all = const.tile([1, UPC, HD + 1], BF16)  # [v_cur | 1]
            nc.scalar.dma_start(
                out=vcur_all[:, :, 0:HD],
                in_=fused_loc[7, :, 2 * HD:3 * HD][None, :, :])
            nc.vector.memset(vcur_all[:, :, HD:HD + 1], 1.0)

            # ---------------- phase C: per-user flash-decode attention ------
            HIDP = KT * 128  # attn padded to 4608 so xbar tiles divide
            attn_c = dram.tile([UPC, HIDP], BF16, name="attn_c")
            zero4 = const.tile([UPC, HD], BF16)
            nc.vector.memset(zero4, 0.0)
            nc.scalar.dma_start(
                out=bass.AP(tensor=attn_c.tensor,
                            offset=attn_c.offset + HID,
                            ap=[[HIDP, UPC], [1, HD]]),
                in_=zero4)
            attn_ag = dram.tile([NCORES, UPC, HIDP], BF16,
                                addr_space="Shared", name="attn_ag")

            for i in range(UPC):
                # [k^T | mask row]: contraction row 64 carries the mask; the
                # q side puts 8.0 there so exp's 1/8 scale yields qk/8 + m
                kT_sb = kpool.tile([HD + 1, S], BF16, tag="kT", name="kT_sb")
                vones = vpool.tile([128, NT, HD + 1], BF16, tag="v",
                                   name="vones")
                if i < 2:
                    # guard: users 0/1 KV loads start only once phase A's
                    # weight traffic is done (fq_sb written), keeping the wq
                    # slabs sole owners of the DMA engines
                    nc.vector.tensor_copy(out=kT_sb[0:1, 0:1],
                                          in_=fq_sb[0:1, 0:1])
                    nc.vector.tensor_copy(out=vones[0:1, 0:1, 0:1],
                                          in_=fq_sb[0:1, 0:1])
                nc.sync.dma_start(out=kT_sb, in_=kTc[i])
                nc.sync.dma_start(out=vones, in_=vc[i])

                # q heads 0..70 plus the shared k head at row 71, transposed
                ps_qT = pstpool.tile([HD, NH + 1], F32, tag="pst",
                                     name="ps_qT")
                nc.tensor.transpose(ps_qT, q_f32[0:NH + 1, i, :],
                                    identity[0:NH + 1, 0:NH + 1])
                qkT = upool.tile([HD, NH + 1], F32, tag="qkT", name="qkT")
                nc.vector.tensor_copy(out=qkT, in_=ps_qT)

                # rotary as a matmul; row 64 = 8.0 scales the kT mask row
                ps_rot = pstpool.tile([HD, NH + 1], F32, tag="pst",
                                      name="ps_rot")
                nc.tensor.matmul(ps_rot, muT_sb[:, i, :], qkT,
                                 start=True, stop=True)
                qTr = upool.tile([HD + 1, NH + 1], BF16, tag="qTr",
                                 name="qTr")
                nc.vector.tensor_copy(out=qTr[0:HD, :], in_=ps_rot)
                nc.vector.memset(qTr[HD:HD + 1, :], 8.0)

                # current-token score for all heads: [1, 71] (no mask row)
                ps_sc = pstpool.tile([1, NH], F32, tag="pst", name="ps_sc")
                nc.tensor.matmul(ps_sc, qTr[0:HD, NH:NH + 1],
                                 qTr[0:HD, 0:NH], start=True, stop=True)
                curw = upool.tile([1, NH], BF16, tag="curw", name="curw")
                nc.scalar.activation(out=curw, in_=ps_sc,
                                     func=mybir.ActivationFunctionType.Exp,
                                     scale=0.125)

                # scores + exp over the 64 s-tiles, 7 tiles per PSUM bank at
                # stride 71, one batched exp per 2-bank group
                pT_all = ppool.tile([128, NT, NH], BF16, tag="pT",
                                    name="pT_all")
                pv = pvpool.tile([NH, HD + 1], F32, tag="pv", name="pv")
                t0 = 0
                for gi, gn in enumerate(EG):
                    ps_g = pscpool.tile([128, 2, 512], F32, tag="sg",
                                        name="ps_g")
                    for k in range(gn):
                        t = t0 + k
                        nc.tensor.matmul(
                            ps_g[:, k // 7, (k % 7) * NH:(k % 7 + 1) * NH],
                            kT_sb[:, t * 128:(t + 1) * 128],
                            qTr[:, 0:NH], start=True, stop=True)
                    if gn == 14:
                        nc.scalar.activation(
                            out=pT_all[:, t0:t0 + 14, :],
                            in_=ps_g[:, :, 0:7 * NH].rearrange(
                                "p b (k h) -> p b k h", h=NH),
                            func=mybir.ActivationFunctionType.Exp,
                            scale=0.125)
                    else:
                        for b in range((gn + 6) // 7):
                            bn = min(7, gn - 7 * b)
                            nc.scalar.activation(
                                out=pT_all[:, t0 + 7 * b:t0 + 7 * b + bn, :],
                                in_=ps_g[:, b, 0:bn * NH].rearrange(
                                    "p (k h) -> p k h", h=NH),
                                func=mybir.ActivationFunctionType.Exp,
                                scale=0.125)
                    t0 += gn

                # PV with fused row-sum via the ones column
                for t in range(NT):
                    nc.tensor.matmul(pv, pT_all[:, t, :], vones[:, t, :],
                                     start=(t == 0), stop=False)
                nc.tensor.matmul(pv, curw, vcur_all[:, i, :], start=False,
                                 stop=True)

                linv = upool.tile([NH, 1], F32, tag="linv", name="linv")
                nc.vector.reciprocal(out=linv, in_=pv[:, HD:HD + 1])
                attn_sb = upool.tile([NH, HD], BF16, tag="attn",
                                     name="attn_sb")
                nc.vector.tensor_scalar_mul(attn_sb, pv[:, 0:HD], linv)
                nc.scalar.dma_start(
                    out=bass.AP(tensor=attn_c.tensor,
                                offset=attn_c.offset + i * HIDP,
                                ap=[[HD, NH], [1, HD]]),
                    in_=attn_sb)

            nc.gpsimd.collective_compute(
                "AllGather", mybir.AluOpType.bypass,
                replica_groups=[list(range(NCORES))],
                ins=[attn_c.opt()], outs=[attn_ag.opt()])

            # ---------------- phase D: dense output projection --------------
            # attnT via one xbar DMA transpose of the gathered activations
            attnT = const.tile([128, KT, U], BF16)
            nc.sync.dma_start_transpose(
                out=attnT, in_=attn_ag.rearrange("c j n -> (c j) n"))

            psD = pqpool.tile([128, DC], F32, tag="bank", name="psD")
            for t in range(KT):
                for j in range(4):
                    nc.tensor.matmul(psD[32 * j:32 * j + 32, :],
                                     attnT[:, t, :],
                                     wd_sb[:, t, DC * j:DC * (j + 1)],
                                     start=(t == 0), stop=(t == KT - 1),
                                     tile_position=(0, 32 * j))

            outD = const.tile([128, DC], F32)
            nc.vector.tensor_copy(out=outD, in_=psD[:, :])
            outc_ji = bass.AP(
                tensor=outc.ap().tensor, offset=0,
                ap=[[DC, 4], [DN, U], [1, DC]])
            nc.scalar.dma_start(out=outc_ji, in_=outD)

    nc.compile()
    return nc


def _rot_mat(cos_u, sin_u):
    """M such that M @ x = x*cos + rotate_half(x)*sin, for one user."""
    m = np.zeros((HD, HD), np.float32)
    np.fill_diagonal(m, cos_u)
    half = HD // 2
    for r in range(half):
        m[r, r + half] += -sin_u[r]
        m[r + half, r] += sin_u[r + half]
    return m


def kernel(hidden_states, cos, sin, k_cache, v_cache, attn_masks, w_qkv,
           w_dense, trace=False):
    global _prog, LAST_RESULT
    import ml_dtypes

    bf16 = ml_dtypes.bfloat16
    if _prog is None:
        _prog = _build()

    hidden_states = np.asarray(hidden_states, np.float32)
    cos = np.asarray(cos, np.float32)
    sin = np.asarray(sin, np.float32)
    k_cache = np.asarray(k_cache, np.float32)
    v_cache = np.asarray(v_cache, np.float32)
    attn_masks = np.asarray(attn_masks, np.float32)
    w_qkv = np.asarray(w_qkv, np.float32)
    w_dense = np.asarray(w_dense, np.float32)

    def pack_k(m, ncol):
        """[4544, ncol] -> [128, 36, ncol] bf16, zero-padded to 4608 rows."""
        p = np.zeros((KT * 128, ncol), np.float32)
        p[:m.shape[0]] = m
        return np.ascontiguousarray(
            p.reshape(KT, 128, ncol).transpose(1, 0, 2).astype(bf16))

    hT = pack_k(hidden_states[0].T, U)                       # [128, 36, 32]
    wqT = np.zeros((HID, NCORES * NCOL), np.float32)
    wqT[:, :w_qkv.shape[0]] = w_qkv.T
    wdT = w_dense.T                                          # [4544, 4544]

    in_maps = []
    for c in range(NCORES):
        us = slice(UPC * c, UPC * (c + 1))
        k_u = np.moveaxis(k_cache[:, 0, us], 1, 0).reshape(UPC, S, HD)
        m_u = np.moveaxis(attn_masks[:, 0, us], 1, 0).reshape(UPC, S)
        kT_u = np.concatenate(
            [np.transpose(k_u, (0, 2, 1)), m_u[:, None, :]], axis=1)
        v_u = np.moveaxis(v_cache[:, 0, us], 1, 0).reshape(UPC, NT, 128, HD)
        vones = np.concatenate(
            [v_u, np.ones((UPC, NT, 128, 1), np.float32)], axis=3)
        muT = np.stack([
            _rot_mat(cos[0, u, 0], sin[0, u, 0]).T
            for u in range(UPC * c, UPC * (c + 1))
        ])                                                   # [4, 64, 64]
        in_maps.append({
            "hT": hT,
            "wq": pack_k(wqT[:, NCOL * c:NCOL * (c + 1)], NCOL),
            "wd": pack_k(wdT[:, DN * c:DN * (c + 1)], DN),
            "kTc": np.ascontiguousarray(kT_u.astype(bf16)),
            "vc": np.ascontiguousarray(
                vones.transpose(0, 2, 1, 3).astype(bf16)),
            "muT": np.ascontiguousarray(
                np.transpose(muT, (1, 0, 2)).astype(np.float32)),
        })

    res = run_bass_kernel_spmd(_prog, in_maps, list(range(NCORES)),
                               trace=trace)
    LAST_RESULT = res
    out = np.concatenate([res.results[c]["outc"] for c in range(NCORES)],
                         axis=1)                             # [32, 4544]
    return out[None].astype(np.float32)


# revision 12
# speedup vs baseline: 2.1148x; 1.0459x over previous
"""Trainium2 Bass kernel for Falcon-7B MQA flash-decode attention block.

Geometry (hardcoded from the problem spec):
  hidden [1, 32, 4544], w_qkv [4672, 4544] (71 q heads + 1 k + 1 v, hd=64),
  kv cache [4, 1, 32, 2048, 64], masks [4, 1, 32, 2048], w_dense [4544, 4544].

Sharding across 8 NeuronCores:
  - users (32) are data-parallel, 4 per core: each core holds its users' KV.
  - w_qkv / w_dense are tensor-parallel column-split 8 ways; an AllToAll
    redistributes the fused QKV activations from column-shards to user-shards,
    one AllGather collects attention outputs for the dense matmul.
  - all matmul operands are bf16 (host-cast); PSUM accumulation stays fp32.
    bf16 is 4x faster on the PE per moving row and halves HBM traffic.
  - softmax uses the shift-invariant (max-free) formulation; the attention
    mask rides along as contraction row 64 of kT with q row 64 = 8.0, so the
    ACT exp's 1/8 scale returns exactly qk/8 + mask. No separate mask ops.
  - scores are packed 7 per PSUM bank at stride 71; exp batches 14 s-tiles
    (2 banks) per ACT op to amortize the ~185ns fixed ACT latency.
  - attnT for the dense matmul comes from a strided DMA load of the gathered
    attention (DRAM->SBUF transposed AP), not PE transposes.

Host-side prep is layout-only (transposes / packing / dtype casts).
"""

import sys

if "/opt/trn_rl_repo" not in sys.path:
    sys.path.insert(0, "/opt/trn_rl_repo")

import numpy as np

import concourse.bacc as bacc
import concourse.bass as bass
import concourse.mybir as mybir
import concourse.tile as tile
from concourse.bass_utils import run_bass_kernel_spmd
from concourse.masks import make_identity

F32 = mybir.dt.float32
BF16 = mybir.dt.bfloat16

NCORES = 8
U = 32          # users total
UPC = 4         # users per core
HID = 4544
NH = 71         # query heads
HD = 64
HPC = 10        # heads per core in the padded qkv column split (8*10*64 = 5120)
NCOL = HPC * HD         # 640 fused columns per core
DN = HID // NCORES      # 568 dense output columns per core
S = 8192                # total cached tokens per user (4 chunks x 2048)
NT = S // 128           # 64 s-tiles of 128
KT = 36                 # k-tiles over HID (zero-padded to 4608 rows)
WQS = 6                 # wq k-tiles per DMA slab (6 slabs of 6)
QC = NCOL // 4          # 160 fused columns per PSUM quadrant
DC = DN // 4            # 142 dense columns per PSUM quadrant
EG = (14, 14, 14, 14, 8)  # exp batch sizes over the 64 s-tiles

LAST_RESULT = None
_prog = None


def _build():
    nc = bacc.Bacc("TRN2", target_bir_lowering=False, debug=False,
                   num_devices=NCORES)

    # host-packed inputs (see kernel() below)
    hT = nc.dram_tensor("hT", [128, KT, U], BF16, kind="ExternalInput")
    wq = nc.dram_tensor("wq", [128, KT, NCOL], BF16, kind="ExternalInput")
    wd = nc.dram_tensor("wd", [128, KT, DN], BF16, kind="ExternalInput")
    kTc = nc.dram_tensor("kTc", [UPC, HD + 1, S], BF16, kind="ExternalInput")
    vc = nc.dram_tensor("vc", [UPC, 128, NT, HD + 1], BF16,
                        kind="ExternalInput")
    muT = nc.dram_tensor("muT", [HD, UPC, HD], F32, kind="ExternalInput")
    outc = nc.dram_tensor("outc", [U, DN], F32, kind="ExternalOutput")

    with tile.TileContext(nc) as tc:
        with (
            tc.tile_pool(name="const", bufs=1) as const,
            tc.tile_pool(name="wpool", bufs=3) as wpool,
            tc.tile_pool(name="kpool", bufs=2) as kpool,
            tc.tile_pool(name="vpool", bufs=2) as vpool,
            tc.tile_pool(name="ppool", bufs=2) as ppool,
            tc.tile_pool(name="upool", bufs=2) as upool,
            tc.tile_pool(name="pqpool", bufs=1, space="PSUM") as pqpool,
            tc.tile_pool(name="psc", bufs=2, space="PSUM") as pscpool,
            tc.tile_pool(name="pvpool", bufs=1, space="PSUM") as pvpool,
            tc.tile_pool(name="pstpool", bufs=2, space="PSUM") as pstpool,
            tc.tile_pool(name="dram", bufs=1, space="DRAM") as dram,
        ):
            identity = const.tile([128, 128], F32)
            make_identity(nc, identity)

            # warm the PE p-state during the initial weight-DMA wait: ~3us of
            # continuous dummy matmuls take the clock 0.65 -> 2.4 GHz before
            # the first real QKV matmul lands
            wtile = const.tile([128, 128], BF16)
            nc.vector.memset(wtile, 0.0)
            ps_w = pscpool.tile([128, 2, 512], F32, tag="sg", name="ps_w")
            for w in range(30):
                nc.tensor.matmul(ps_w[0:1, 0, 0:128], wtile[:, 0:1],
                                 wtile[:, 0:128], start=True, stop=True)

            # ---------------- phase A: fused QKV projection ----------------
            hT_all = const.tile([128, KT, U], BF16)
            nc.sync.dma_start(out=hT_all, in_=hT[:, :, :])
            muT_sb = const.tile([HD, UPC, HD], F32)
            nc.scalar.dma_start(out=muT_sb, in_=muT[:, :, :])

            psQ = pqpool.tile([128, QC], F32, tag="bank", name="psQ")
            wslabs = []
            for g in range(WQS):
                wslab = wpool.tile([128, WQS, NCOL], BF16, tag="w",
                                   name="wslab", uniquify=True)
                if g == 0:
                    # split the first slab so the projection starts after two
                    # k-tiles instead of the full slab
                    nc.sync.dma_start(out=wslab[:, 0:2, :],
                                      in_=wq[:, 0:2, :])
                    nc.sync.dma_start(out=wslab[:, 2:WQS, :],
                                      in_=wq[:, 2:WQS, :])
                else:
                    nc.sync.dma_start(out=wslab,
                                      in_=wq[:, WQS * g:WQS * (g + 1), :])
                wslabs.append(wslab)
            for g in range(WQS):
                for t6 in range(WQS):
                    t = WQS * g + t6
                    lhs = hT_all[:, t, :]
                    for j in range(4):
                        nc.tensor.matmul(
                            psQ[32 * j:32 * j + 32, :], lhs,
                            wslabs[g][:, t6, QC * j:QC * (j + 1)],
                            start=(t == 0), stop=(t == KT - 1),
                            tile_position=(0, 32 * j))

            fq_sb = const.tile([128, QC], BF16)
            nc.scalar.activation(out=fq_sb, in_=psQ[:, :],
                                 func=mybir.ActivationFunctionType.Copy)

            fused_x = dram.tile([U, NCOL], BF16)
            fused_x_ji = bass.AP(
                tensor=fused_x.tensor, offset=fused_x.offset,
                ap=[[QC, 4], [NCOL, U], [1, QC]])
            # ACT-ring store: the SP ring is busy prefetching KV
            nc.scalar.dma_start(out=fused_x_ji, in_=fq_sb)
            # block d of the flat input (users 4d..4d+3) goes to core d
            fused_loc = dram.tile([NCORES, UPC, NCOL], BF16)
            nc.gpsimd.collective_compute(
                "AllToAll", mybir.AluOpType.bypass,
                replica_groups=[list(range(NCORES))],
                ins=[fused_x.opt()], outs=[fused_loc.opt()])

            # one strided gather for all 4 local users: (head, user, d)
            q_bf = const.tile([80, UPC, HD], BF16)
            for i in range(UPC):
                nc.scalar.dma_start(
                    out=q_bf[:, i, :],
                    in_=bass.AP(
                        tensor=fused_loc.tensor,
                        offset=fused_loc.offset + i * NCOL,
                        ap=[[UPC * NCOL, NCORES], [HD, HPC], [1, HD]]))
            q_f32 = const.tile([NH + 1, UPC, HD], F32)
            for i in range(UPC):
                nc.vector.tensor_copy(out=q_f32[:, i, :],
                                      in_=q_bf[0:NH + 1, i, :])
            wd_sb = const.tile([128, KT, DN], BF16)
            # dense-weight prefetch, interleaved with the KV stream: slabs
            # 0/1 go behind the last q gather (after the tiny critical
            # gathers, before user-2 KV), slabs 2/3 behind user-3's KV
            for g in range(2):
                nc.vector.tensor_copy(
                    out=wd_sb[0:1, 9 * g:9 * g + 1, 0:1],
                    in_=q_bf[0:1, UPC - 1, 0:1])
                nc.sync.dma_start(
                    out=wd_sb[:, 9 * g:9 * (g + 1), :],
                    in_=wd[:, 9 * g:9 * (g + 1), :])
            vcur_all = const.tile([1, UPC, HD + 1], BF16)  # [v_cur | 1]
            nc.scalar.dma_start(
                out=vcur_all[:, :, 0:HD],
                in_=fused_loc[7, :, 2 * HD:3 * HD][None, :, :])
            nc.vector.memset(vcur_all[:, :, HD:HD + 1], 1.0)

            # ---------------- phase C: per-user flash-decode attention ------
            HIDP = KT * 128  # attn padded to 4608 so xbar tiles divide
            attn_c = dram.tile([UPC, HIDP], BF16, name="attn_c")
            zero4 = const.tile([UPC, HD], BF16)
            nc.vector.memset(zero4, 0.0)
            nc.scalar.dma_start(
                out=bass.AP(tensor=attn_c.tensor,
                            offset=attn_c.offset + HID,
                            ap=[[HIDP, UPC], [1, HD]]),
                in_=zero4)
            attn_ag = dram.tile([NCORES, UPC, HIDP], BF16,
                                addr_space="Shared", name="attn_ag")

            for i in range(UPC):
                # [k^T | mask row]: contraction row 64 carries the mask; the
                # q side puts 8.0 there so exp's 1/8 scale yields qk/8 + m
                kT_sb = kpool.tile([HD + 1, S], BF16, tag="kT", name="kT_sb")
                vones = vpool.tile([128, NT, HD + 1], BF16, tag="v",
                                   name="vones")
                if i < 2:
                    # guard: users 0/1 KV loads start only once phase A's
                    # weight traffic is done (fq_sb written), keeping the wq
                    # slabs sole owners of the DMA engines
                    nc.vector.tensor_copy(out=kT_sb[0:1, 0:1],
                                          in_=fq_sb[0:1, 0:1])
                    nc.vector.tensor_copy(out=vones[0:1, 0:1, 0:1],
                                          in_=fq_sb[0:1, 0:1])
                nc.sync.dma_start(out=kT_sb, in_=kTc[i])
                nc.sync.dma_start(out=vones, in_=vc[i])
                if i == UPC - 1:
                    for g in range(2, 4):
                        nc.vector.tensor_copy(
                            out=wd_sb[0:1, 9 * g:9 * g + 1, 0:1],
                            in_=vones[0:1, 0:1, 0:1])
                        nc.sync.dma_start(
                            out=wd_sb[:, 9 * g:9 * (g + 1), :],
                            in_=wd[:, 9 * g:9 * (g + 1), :])

                # q heads 0..70 plus the shared k head at row 71, transposed
                ps_qT = pstpool.tile([HD, NH + 1], F32, tag="pst",
                                     name="ps_qT")
                nc.tensor.transpose(ps_qT, q_f32[0:NH + 1, i, :],
                                    identity[0:NH + 1, 0:NH + 1])
                qkT = upool.tile([HD, NH + 1], F32, tag="qkT", name="qkT")
                nc.vector.tensor_copy(out=qkT, in_=ps_qT)

                # rotary as a matmul; row 64 = 8.0 scales the kT mask row
                ps_rot = pstpool.tile([HD, NH + 1], F32, tag="pst",
                                      name="ps_rot")
                nc.tensor.matmul(ps_rot, muT_sb[:, i, :], qkT,
                                 start=True, stop=True)
                qTr = upool.tile([HD + 1, NH + 1], BF16, tag="qTr",
                                 name="qTr")
                nc.vector.tensor_copy(out=qTr[0:HD, :], in_=ps_rot)
                nc.vector.memset(qTr[HD:HD + 1, :], 8.0)

                # current-token score for all heads: [1, 71] (no mask row)
                ps_sc = pstpool.tile([1, NH], F32, tag="pst", name="ps_sc")
                nc.tensor.matmul(ps_sc, qTr[0:HD, NH:NH + 1],
                                 qTr[0:HD, 0:NH], start=True, stop=True)
                curw = upool.tile([1, NH], BF16, tag="curw", name="curw")
                nc.scalar.activation(out=curw, in_=ps_sc,
                                     func=mybir.ActivationFunctionType.Exp,
                                     scale=0.125)

                # scores + exp over the 64 s-tiles, 7 tiles per PSUM bank at
                # stride 71, one batched exp per 2-bank group
                pT_all = ppool.tile([128, NT, NH], BF16, tag="pT",
                                    name="pT_all")
                pv = pvpool.tile([NH, HD + 1], F32, tag="pv", name="pv")
                t0 = 0
                for gi, gn in enumerate(EG):
                    ps_g = pscpool.tile([128, 2, 512], F32, tag="sg",
                                        name="ps_g")
                    for k in range(gn):
                        t = t0 + k
                        nc.tensor.matmul(
                            ps_g[:, k // 7, (k % 7) * NH:(k % 7 + 1) * NH],
                            kT_sb[:, t * 128:(t + 1) * 128],
                            qTr[:, 0:NH], start=True, stop=True)
                    if gn == 14:
                        nc.scalar.activation(
                            out=pT_all[:, t0:t0 + 14, :],
                            in_=ps_g[:, :, 0:7 * NH].rearrange(
                                "p b (k h) -> p b k h", h=NH),
                            func=mybir.ActivationFunctionType.Exp,
                            scale=0.125)
                    else:
                        for b in range((gn + 6) // 7):
                            bn = min(7, gn - 7 * b)
                            nc.scalar.activation(
                                out=pT_all[:, t0 + 7 * b:t0 + 7 * b + bn, :],
                                in_=ps_g[:, b, 0:bn * NH].rearrange(
                                    "p (k h) -> p k h", h=NH),
                                func=mybir.ActivationFunctionType.Exp,
                                scale=0.125)
                    t0 += gn

                # PV with fused row-sum via the ones column
                for t in range(NT):
                    nc.tensor.matmul(pv, pT_all[:, t, :], vones[:, t, :],
                                     start=(t == 0), stop=False)
                nc.tensor.matmul(pv, curw, vcur_all[:, i, :], start=False,
                                 stop=True)

                linv = upool.tile([NH, 1], F32, tag="linv", name="linv")
                nc.vector.reciprocal(out=linv, in_=pv[:, HD:HD + 1])
                attn_sb = upool.tile([NH, HD], BF16, tag="attn",
                                     name="attn_sb")
                nc.vector.tensor_scalar_mul(attn_sb, pv[:, 0:HD], linv)
                nc.scalar.dma_start(
                    out=bass.AP(tensor=attn_c.tensor,
                                offset=attn_c.offset + i * HIDP,
                                ap=[[HD, NH], [1, HD]]),
                    in_=attn_sb)

            nc.gpsimd.collective_compute(
                "AllGather", mybir.AluOpType.bypass,
                replica_groups=[list(range(NCORES))],
                ins=[attn_c.opt()], outs=[attn_ag.opt()])

            # ---------------- phase D: dense output projection --------------
            # attnT via one xbar DMA transpose of the gathered activations
            attnT = const.tile([128, KT, U], BF16)
            nc.sync.dma_start_transpose(
                out=attnT, in_=attn_ag.rearrange("c j n -> (c j) n"))

            psD = pqpool.tile([128, DC], F32, tag="bank", name="psD")
            for t in range(KT):
                for j in range(4):
                    nc.tensor.matmul(psD[32 * j:32 * j + 32, :],
                                     attnT[:, t, :],
                                     wd_sb[:, t, DC * j:DC * (j + 1)],
                                     start=(t == 0), stop=(t == KT - 1),
                                     tile_position=(0, 32 * j))

            outD = const.tile([128, DC], F32)
            nc.vector.tensor_copy(out=outD, in_=psD[:, :])
            outc_ji = bass.AP(
                tensor=outc.ap().tensor, offset=0,
                ap=[[DC, 4], [DN, U], [1, DC]])
            nc.scalar.dma_start(out=outc_ji, in_=outD)

    nc.compile()
    return nc


def _rot_mat(cos_u, sin_u):
    """M such that M @ x = x*cos + rotate_half(x)*sin, for one user."""
    m = np.zeros((HD, HD), np.float32)
    np.fill_diagonal(m, cos_u)
    half = HD // 2
    for r in range(half):
        m[r, r + half] += -sin_u[r]
        m[r + half, r] += sin_u[r + half]
    return m


def kernel(hidden_states, cos, sin, k_cache, v_cache, attn_masks, w_qkv,
           w_dense, trace=False):
    global _prog, LAST_RESULT
    import ml_dtypes

    bf16 = ml_dtypes.bfloat16
    if _prog is None:
        _prog = _build()

    hidden_states = np.asarray(hidden_states, np.float32)
    cos = np.asarray(cos, np.float32)
    sin = np.asarray(sin, np.float32)
    k_cache = np.asarray(k_cache, np.float32)
    v_cache = np.asarray(v_cache, np.float32)
    attn_masks = np.asarray(attn_masks, np.float32)
    w_qkv = np.asarray(w_qkv, np.float32)
    w_dense = np.asarray(w_dense, np.float32)

    def pack_k(m, ncol):
        """[4544, ncol] -> [128, 36, ncol] bf16, zero-padded to 4608 rows."""
        p = np.zeros((KT * 128, ncol), np.float32)
        p[:m.shape[0]] = m
        return np.ascontiguousarray(
            p.reshape(KT, 128, ncol).transpose(1, 0, 2).astype(bf16))

    hT = pack_k(hidden_states[0].T, U)                       # [128, 36, 32]
    wqT = np.zeros((HID, NCORES * NCOL), np.float32)
    wqT[:, :w_qkv.shape[0]] = w_qkv.T
    wdT = w_dense.T                                          # [4544, 4544]

    in_maps = []
    for c in range(NCORES):
        us = slice(UPC * c, UPC * (c + 1))
        k_u = np.moveaxis(k_cache[:, 0, us], 1, 0).reshape(UPC, S, HD)
        m_u = np.moveaxis(attn_masks[:, 0, us], 1, 0).reshape(UPC, S)
        kT_u = np.concatenate(
            [np.transpose(k_u, (0, 2, 1)), m_u[:, None, :]], axis=1)
        v_u = np.moveaxis(v_cache[:, 0, us], 1, 0).reshape(UPC, NT, 128, HD)
        vones = np.concatenate(
            [v_u, np.ones((UPC, NT, 128, 1), np.float32)], axis=3)
        muT = np.stack([
            _rot_mat(cos[0, u, 0], sin[0, u, 0]).T
            for u in range(UPC * c, UPC * (c + 1))
        ])                                                   # [4, 64, 64]
        in_maps.append({
            "hT": hT,
            "wq": pack_k(wqT[:, NCOL * c:NCOL * (c + 1)], NCOL),
            "wd": pack_k(wdT[:, DN * c:DN * (c + 1)], DN),
            "kTc": np.ascontiguousarray(kT_u.astype(bf16)),
            "vc": np.ascontiguousarray(
                vones.transpose(0, 2, 1, 3).astype(bf16)),
            "muT": np.ascontiguousarray(
                np.transpose(muT, (1, 0, 2)).astype(np.float32)),
        })

    res = run_bass_kernel_spmd(_prog, in_maps, list(range(NCORES)),
                               trace=trace)
    LAST_RESULT = res
    out = np.concatenate([res.results[c]["outc"] for c in range(NCORES)],
                         axis=1)                             # [32, 4544]
    return out[None].astype(np.float32)
